# revision 2
# baseline (speedup 1.0000x reference)
"""Dynamic per-sample CNN (nn_ConvFunc) Trainium2 Bass kernel — bf16 version.

Reference computation (per sample b):
  cnn_inp = proj_w @ cat(lhs, rhs) + proj_b          # 1x1 conv, [128, 32, 32]
  out     = conv3x3(cnn_inp, W_b) + bias_b           # W_b, bias_b unpacked from question_rep[b]

Sharding: pure data parallel, 8 samples per NeuronCore (batch 64 / 8 cores).

v2 changes vs the fp32r baseline (67.9us):
  - all matmul operands bf16 (tolerance 2e-2 >> bf16's ~2e-3): halves HBM
    traffic (17.3MB -> 8.6MB per core), moving the kernel from DMA-bound
    (48us floor) to PE-bound (37.5us floor = 90112 PE cycles @ 2.4GHz).
  - per-sample inputs packed host-side into one [128, 1152+2048] tile
    ([qw | xl | xr]) so loads are few large DMAs; all loads issued up-front
    (SBUF holds all 8 samples), ordered x0,x1,qw0,x2,qw1,... so the PE
    pipeline never starves during ramp-up.
  - dummy warmup matmuls during the initial DMA ramp lift the PE HAM clock
    gate (4/8 -> 8/8 after ~3.4us of activity).
  - conv PSUM eviction split across DVE (half 0) and ACT (half 1); output
    stored as bf16 (host upcasts) halving store traffic and eviction work.
"""

import numpy as np
import ml_dtypes

import concourse.bass as bass
import concourse.mybir as mybir
from concourse import bacc
from concourse.tile import TileContext
from concourse.bass_utils import run_bass_kernel_spmd

# Problem shapes (hardcoded per contract)
B = 64
DIM = 128
H = W = 32
K = 3
KK = K * K
HW = H * W             # 1024
WDIM = DIM * DIM * KK  # 147456
NCORES = 8
SPC = B // NCORES      # samples per core
HP, WP = H + 2, W + 2  # padded 34x34
HALF = HW // 2         # 512 columns per PSUM bank
HROWS = H // 2         # 16 output rows per half
QWC = KK * DIM         # 1152 conv-weight cols per sample
PXC = QWC + 2 * HW     # 3200 packed cols per sample: [qw | xl | xr]

FP = mybir.dt.float32
BF = mybir.dt.bfloat16
BF_NP = ml_dtypes.bfloat16

_BUILT = {}


def build_nc():
    nc = bacc.Bacc("TRN2", target_bir_lowering=False, debug=False,
                   num_devices=NCORES)

    px = nc.declare_dram_parameter("px", [SPC, DIM, PXC], BF, isOutput=False)
    pw = nc.declare_dram_parameter("pw", [DIM, 2 * DIM], BF, isOutput=False)
    bias = nc.declare_dram_parameter("bias", [DIM, SPC + 1], FP, isOutput=False)
    out = nc.declare_dram_parameter("out", [SPC, DIM, HW], BF, isOutput=True)

    with TileContext(nc) as tc:
        with (
            tc.tile_pool(name="const", bufs=1) as cpool,
            tc.tile_pool(name="pxpool", bufs=SPC) as pxpool,
            tc.tile_pool(name="xppool", bufs=4) as xppool,
            tc.tile_pool(name="opool", bufs=4) as opool,
            tc.tile_pool(name="pp_pool", bufs=2, space="PSUM") as pp_pool,
            tc.tile_pool(name="pc_pool", bufs=4, space="PSUM") as pc_pool,
            tc.tile_pool(name="wm_pool", bufs=1, space="PSUM") as wm_pool,
        ):
            pw_sb = cpool.tile([DIM, 2 * DIM], BF)
            bias_sb = cpool.tile([DIM, SPC + 1], FP)
            warm = cpool.tile([DIM, HALF], BF)
            nc.vector.memset(warm[:], 0.0)
            nc.sync.dma_start(out=pw_sb[:], in_=pw[:])
            nc.sync.dma_start(out=bias_sb[:], in_=bias[:])

            # ---- all input loads up-front, latency-ordered ----------------
            px_sb = [pxpool.tile([DIM, PXC], BF, tag="px") for _ in range(SPC)]

            def load_x(s):
                if s == 0:
                    # land operands in the order proj(0)'s matmuls need them
                    for h in range(2):
                        for o in (QWC, QWC + HW):  # xl half, xr half
                            nc.sync.dma_start(
                                out=px_sb[s][:, o + h * HALF:o + (h + 1) * HALF],
                                in_=px[s, :, o + h * HALF:o + (h + 1) * HALF])
                else:
                    nc.sync.dma_start(out=px_sb[s][:, QWC:PXC],
                                      in_=px[s, :, QWC:PXC])

            def load_w(s):
                nc.sync.dma_start(out=px_sb[s][:, 0:QWC], in_=px[s, :, 0:QWC])

            load_x(0)
            load_x(1)
            load_w(0)
            for s in range(2, SPC):
                load_x(s)
                load_w(s - 1)
            load_w(SPC - 1)

            # ---- PE warmup: lift the HAM clock gate during the DMA ramp ---
            wmb = wm_pool.tile([DIM, HALF], FP)
            for _ in range(5):
                nc.tensor.matmul(wmb[:], lhsT=warm[:, 0:DIM], rhs=warm[:],
                                 start=True, stop=True)

            pb_ap = bias_sb[:, SPC:SPC + 1]

            def proj(s):
                xp = xppool.tile([DIM, HP, WP], BF, tag="xp")
                nc.vector.memset(xp[:, 0:1, :], 0.0)
                nc.vector.memset(xp[:, HP - 1:HP, :], 0.0)
                nc.vector.memset(xp[:, 1:HP - 1, 0:1], 0.0)
                nc.vector.memset(xp[:, 1:HP - 1, WP - 1:WP], 0.0)
                for h in range(2):
                    ppt = pp_pool.tile([DIM, HALF], FP, tag="pp")
                    nc.tensor.matmul(ppt[:], lhsT=pw_sb[:, 0:DIM],
                                     rhs=px_sb[s][:, QWC + h * HALF:
                                                  QWC + (h + 1) * HALF],
                                     start=True, stop=False)
                    nc.tensor.matmul(ppt[:], lhsT=pw_sb[:, DIM:2 * DIM],
                                     rhs=px_sb[s][:, QWC + HW + h * HALF:
                                                  QWC + HW + (h + 1) * HALF],
                                     start=False, stop=True)
                    nc.scalar.activation(
                        xp[:, 1 + HROWS * h:1 + HROWS * (h + 1), 1:1 + W],
                        ppt[:].rearrange("p (a b) -> p a b", b=W),
                        mybir.ActivationFunctionType.Identity,
                        bias=pb_ap,
                    )
                return xp

            def conv(s, xp):
                o_sb = opool.tile([DIM, HW], BF, tag="o")
                pct0 = pc_pool.tile([DIM, HALF], FP, tag="pc")
                pct1 = pc_pool.tile([DIM, HALF], FP, tag="pc")
                pcts = [pct0, pct1]
                qb = bias_sb[:, s:s + 1]
                # tap-outer: consecutive matmuls share the stationary weights
                for t in range(KK):
                    kh, kw = divmod(t, K)
                    for h in range(2):
                        nc.tensor.matmul(
                            pcts[h][:],
                            lhsT=px_sb[s][:, t * DIM:(t + 1) * DIM],
                            rhs=xp[:, HROWS * h + kh:HROWS * (h + 1) + kh,
                                   kw:kw + W],
                            start=(t == 0), stop=(t == KK - 1))
                # evict half 0 on DVE, half 1 on ACT; bf16 out halves traffic
                nc.vector.tensor_scalar_add(o_sb[:, 0:HALF], pct0[:], qb)
                nc.scalar.activation(
                    o_sb[:, HALF:HW], pct1[:],
                    mybir.ActivationFunctionType.Identity, bias=qb)
                if s == SPC - 1:
                    nc.scalar.dma_start(out=out[s, :, 0:HALF],
                                        in_=o_sb[:, 0:HALF])
                    nc.scalar.dma_start(out=out[s, :, HALF:HW],
                                        in_=o_sb[:, HALF:HW])
                else:
                    nc.scalar.dma_start(out=out[s], in_=o_sb[:])

            # software pipeline: proj(s) ahead of conv(s-1) keeps PE dense
            prev = None
            for s in range(SPC):
                xp = proj(s)
                if prev is not None:
                    conv(*prev)
                prev = (s, xp)
            conv(*prev)

    nc.compile()
    return nc


def _prep(question_rep, lhs_rep, rhs_rep, proj_w, proj_b):
    """Host-side shard + layout prep (reshape/transpose + bf16 cast)."""
    qr = np.ascontiguousarray(question_rep, dtype=np.float32)
    # conv weights: [B, o, i, kh, kw] -> [B, i, (kh kw), o] so each tap is a
    # ready lhsT [i, o] block
    qw = qr[:, :WDIM].reshape(B, DIM, DIM, K, K).transpose(0, 2, 3, 4, 1)
    qw = np.ascontiguousarray(qw).reshape(B, DIM, QWC)
    qb = np.ascontiguousarray(qr[:, WDIM:])             # [B, 128]
    xl = np.asarray(lhs_rep, dtype=np.float32).reshape(B, DIM, HW)
    xr = np.asarray(rhs_rep, dtype=np.float32).reshape(B, DIM, HW)
    pxf = np.concatenate([qw, xl, xr], axis=2).astype(BF_NP)  # [B, 128, 3200]
    pwt = np.asarray(proj_w, dtype=np.float32).T        # [256, 128]
    pw_h = np.concatenate([pwt[:DIM], pwt[DIM:]], axis=1).astype(BF_NP)
    pb = np.asarray(proj_b, dtype=np.float32).reshape(DIM, 1)

    in_maps = []
    for c in range(NCORES):
        sl = slice(c * SPC, (c + 1) * SPC)
        biasm = np.concatenate([qb[sl].T, pb], axis=1).astype(np.float32)
        in_maps.append({
            "px": np.ascontiguousarray(pxf[sl]),
            "pw": pw_h,
            "bias": np.ascontiguousarray(biasm),
        })
    return in_maps


def kernel(question_rep, lhs_rep, rhs_rep, proj_w, proj_b, _run_kwargs=None):
    if "nc" not in _BUILT:
        _BUILT["nc"] = build_nc()
    nc = _BUILT["nc"]
    in_maps = _prep(question_rep, lhs_rep, rhs_rep, proj_w, proj_b)
    res = run_bass_kernel_spmd(nc, in_maps, core_ids=list(range(NCORES)),
                               **(_run_kwargs or {}))
    out = np.concatenate([np.asarray(res.results[c]["out"])
                          for c in range(NCORES)], axis=0)
    if _run_kwargs is not None:
        _BUILT["last_result"] = res
    return out.astype(np.float32).reshape(B, DIM, H, W)


if __name__ == "__main__":
    rng = np.random.default_rng(0)
    inputs = {
        "question_rep": rng.standard_normal((B, WDIM + DIM), dtype=np.float32) * 0.05,
        "lhs_rep": rng.standard_normal((B, DIM, H, W), dtype=np.float32),
        "rhs_rep": rng.standard_normal((B, DIM, H, W), dtype=np.float32),
        "proj_w": rng.standard_normal((DIM, 2 * DIM), dtype=np.float32),
        "proj_b": rng.standard_normal((DIM,), dtype=np.float32) * 0.01,
    }
    out = kernel(**inputs)
    print("ran, out shape:", out.shape)


# revision 3
# speedup vs baseline: 1.1279x; 1.1279x over previous
"""Dynamic per-sample CNN (nn_ConvFunc) Trainium2 Bass kernel — bf16 version.

Reference computation (per sample b):
  cnn_inp = proj_w @ cat(lhs, rhs) + proj_b          # 1x1 conv, [128, 32, 32]
  out     = conv3x3(cnn_inp, W_b) + bias_b           # W_b, bias_b unpacked from question_rep[b]

Sharding: pure data parallel, 8 samples per NeuronCore (batch 64 / 8 cores).

v2 changes vs the fp32r baseline (67.9us):
  - all matmul operands bf16 (tolerance 2e-2 >> bf16's ~2e-3): halves HBM
    traffic (17.3MB -> 8.6MB per core), moving the kernel from DMA-bound
    (48us floor) to PE-bound (37.5us floor = 90112 PE cycles @ 2.4GHz).
  - per-sample inputs packed host-side into one [128, 1152+2048] tile
    ([qw | xl | xr]) so loads are few large DMAs; all loads issued up-front
    (SBUF holds all 8 samples), ordered x0,x1,qw0,x2,qw1,... so the PE
    pipeline never starves during ramp-up.
  - dummy warmup matmuls during the initial DMA ramp lift the PE HAM clock
    gate (4/8 -> 8/8 after ~3.4us of activity).
  - conv PSUM eviction split across DVE (half 0) and ACT (half 1); output
    stored as bf16 (host upcasts) halving store traffic and eviction work.
"""

import numpy as np
import ml_dtypes

import concourse.bass as bass
import concourse.mybir as mybir
from concourse import bacc
from concourse.tile import TileContext
from concourse.bass_utils import run_bass_kernel_spmd

# Problem shapes (hardcoded per contract)
B = 64
DIM = 128
H = W = 32
K = 3
KK = K * K
HW = H * W             # 1024
WDIM = DIM * DIM * KK  # 147456
NCORES = 8
SPC = B // NCORES      # samples per core
HP, WP = H + 2, W + 2  # padded 34x34
HALF = HW // 2         # 512 columns per PSUM bank
HROWS = H // 2         # 16 output rows per half
QWC = KK * DIM         # 1152 conv-weight cols per sample
PXC = QWC + 2 * HW     # 3200 packed cols per sample: [qw | xl | xr]

FP = mybir.dt.float32
BF = mybir.dt.bfloat16
BF_NP = ml_dtypes.bfloat16

_BUILT = {}


def build_nc():
    nc = bacc.Bacc("TRN2", target_bir_lowering=False, debug=False,
                   num_devices=NCORES)

    px = nc.declare_dram_parameter("px", [SPC, DIM, PXC], BF, isOutput=False)
    pw = nc.declare_dram_parameter("pw", [DIM, 2 * DIM], BF, isOutput=False)
    bias = nc.declare_dram_parameter("bias", [DIM, SPC + 1], FP, isOutput=False)
    out = nc.declare_dram_parameter("out", [SPC, DIM, HW], BF, isOutput=True)

    with TileContext(nc) as tc:
        with (
            tc.tile_pool(name="const", bufs=1) as cpool,
            tc.tile_pool(name="pxpool", bufs=SPC) as pxpool,
            tc.tile_pool(name="xppool", bufs=4) as xppool,
            tc.tile_pool(name="opool", bufs=4) as opool,
            tc.tile_pool(name="pp_pool", bufs=2, space="PSUM") as pp_pool,
            tc.tile_pool(name="pc_pool", bufs=4, space="PSUM") as pc_pool,
            tc.tile_pool(name="wm_pool", bufs=1, space="PSUM") as wm_pool,
        ):
            pw_sb = cpool.tile([DIM, 2 * DIM], BF)
            bias_sb = cpool.tile([DIM, SPC + 1], FP)
            warm = cpool.tile([DIM, HALF], BF)
            nc.vector.memset(warm[:], 0.0)
            nc.sync.dma_start(out=pw_sb[:], in_=pw[:])
            nc.sync.dma_start(out=bias_sb[:], in_=bias[:])

            # ---- all input loads up-front, latency-ordered ----------------
            px_sb = [pxpool.tile([DIM, PXC], BF, tag="px", name=f"px{s}")
                     for s in range(SPC)]

            def load_x(s):
                if s == 0:
                    # land operands in the order proj(0)'s matmuls need them
                    for h in range(2):
                        for o in (QWC, QWC + HW):  # xl half, xr half
                            nc.sync.dma_start(
                                out=px_sb[s][:, o + h * HALF:o + (h + 1) * HALF],
                                in_=px[s, :, o + h * HALF:o + (h + 1) * HALF])
                else:
                    nc.sync.dma_start(out=px_sb[s][:, QWC:PXC],
                                      in_=px[s, :, QWC:PXC])

            def load_w(s):
                nc.sync.dma_start(out=px_sb[s][:, 0:QWC], in_=px[s, :, 0:QWC])

            load_x(0)
            load_x(1)
            load_w(0)
            for s in range(2, SPC):
                load_x(s)
                load_w(s - 1)
            load_w(SPC - 1)

            # ---- PE warmup: lift the HAM clock gate during the DMA ramp ---
            wmb = wm_pool.tile([DIM, HALF], FP)
            for _ in range(5):
                nc.tensor.matmul(wmb[:], lhsT=warm[:, 0:DIM], rhs=warm[:],
                                 start=True, stop=True)

            pb_ap = bias_sb[:, SPC:SPC + 1]

            def proj(s):
                xp = xppool.tile([DIM, HP, WP], BF, tag="xp")
                nc.vector.memset(xp[:, 0:1, :], 0.0)
                nc.vector.memset(xp[:, HP - 1:HP, :], 0.0)
                nc.vector.memset(xp[:, 1:HP - 1, 0:1], 0.0)
                nc.vector.memset(xp[:, 1:HP - 1, WP - 1:WP], 0.0)
                for h in range(2):
                    ppt = pp_pool.tile([DIM, HALF], FP, tag="pp")
                    nc.tensor.matmul(ppt[:], lhsT=pw_sb[:, 0:DIM],
                                     rhs=px_sb[s][:, QWC + h * HALF:
                                                  QWC + (h + 1) * HALF],
                                     start=True, stop=False)
                    nc.tensor.matmul(ppt[:], lhsT=pw_sb[:, DIM:2 * DIM],
                                     rhs=px_sb[s][:, QWC + HW + h * HALF:
                                                  QWC + HW + (h + 1) * HALF],
                                     start=False, stop=True)
                    nc.scalar.activation(
                        xp[:, 1 + HROWS * h:1 + HROWS * (h + 1), 1:1 + W],
                        ppt[:].rearrange("p (a b) -> p a b", b=W),
                        mybir.ActivationFunctionType.Identity,
                        bias=pb_ap,
                    )
                return xp

            def conv(s, xp):
                o_sb = opool.tile([DIM, HW], BF, tag="o")
                pct0 = pc_pool.tile([DIM, HALF], FP, tag="pc")
                pct1 = pc_pool.tile([DIM, HALF], FP, tag="pc")
                pcts = [pct0, pct1]
                qb = bias_sb[:, s:s + 1]
                # tap-outer: consecutive matmuls share the stationary weights
                for t in range(KK):
                    kh, kw = divmod(t, K)
                    for h in range(2):
                        nc.tensor.matmul(
                            pcts[h][:],
                            lhsT=px_sb[s][:, t * DIM:(t + 1) * DIM],
                            rhs=xp[:, HROWS * h + kh:HROWS * (h + 1) + kh,
                                   kw:kw + W],
                            start=(t == 0), stop=(t == KK - 1))
                # evict half 0 on DVE, half 1 on ACT; bf16 out halves traffic
                nc.vector.tensor_scalar_add(o_sb[:, 0:HALF], pct0[:], qb)
                nc.scalar.activation(
                    o_sb[:, HALF:HW], pct1[:],
                    mybir.ActivationFunctionType.Identity, bias=qb)
                if s == SPC - 1:
                    nc.scalar.dma_start(out=out[s, :, 0:HALF],
                                        in_=o_sb[:, 0:HALF])
                    nc.scalar.dma_start(out=out[s, :, HALF:HW],
                                        in_=o_sb[:, HALF:HW])
                else:
                    nc.scalar.dma_start(out=out[s], in_=o_sb[:])

            # software pipeline: proj(s) ahead of conv(s-1) keeps PE dense
            prev = None
            for s in range(SPC):
                xp = proj(s)
                if prev is not None:
                    conv(*prev)
                prev = (s, xp)
            conv(*prev)

    nc.compile()
    return nc


def _prep(question_rep, lhs_rep, rhs_rep, proj_w, proj_b):
    """Host-side shard + layout prep (reshape/transpose + bf16 cast)."""
    qr = np.ascontiguousarray(question_rep, dtype=np.float32)
    # conv weights: [B, o, i, kh, kw] -> [B, i, (kh kw), o] so each tap is a
    # ready lhsT [i, o] block
    qw = qr[:, :WDIM].reshape(B, DIM, DIM, K, K).transpose(0, 2, 3, 4, 1)
    qw = np.ascontiguousarray(qw).reshape(B, DIM, QWC)
    qb = np.ascontiguousarray(qr[:, WDIM:])             # [B, 128]
    xl = np.asarray(lhs_rep, dtype=np.float32).reshape(B, DIM, HW)
    xr = np.asarray(rhs_rep, dtype=np.float32).reshape(B, DIM, HW)
    pxf = np.concatenate([qw, xl, xr], axis=2).astype(BF_NP)  # [B, 128, 3200]
    pwt = np.asarray(proj_w, dtype=np.float32).T        # [256, 128]
    pw_h = np.concatenate([pwt[:DIM], pwt[DIM:]], axis=1).astype(BF_NP)
    pb = np.asarray(proj_b, dtype=np.float32).reshape(DIM, 1)

    in_maps = []
    for c in range(NCORES):
        sl = slice(c * SPC, (c + 1) * SPC)
        biasm = np.concatenate([qb[sl].T, pb], axis=1).astype(np.float32)
        in_maps.append({
            "px": np.ascontiguousarray(pxf[sl]),
            "pw": pw_h,
            "bias": np.ascontiguousarray(biasm),
        })
    return in_maps


def kernel(question_rep, lhs_rep, rhs_rep, proj_w, proj_b, _run_kwargs=None):
    if "nc" not in _BUILT:
        _BUILT["nc"] = build_nc()
    nc = _BUILT["nc"]
    in_maps = _prep(question_rep, lhs_rep, rhs_rep, proj_w, proj_b)
    res = run_bass_kernel_spmd(nc, in_maps, core_ids=list(range(NCORES)),
                               **(_run_kwargs or {}))
    out = np.concatenate([np.asarray(res.results[c]["out"])
                          for c in range(NCORES)], axis=0)
    if _run_kwargs is not None:
        _BUILT["last_result"] = res
    return out.astype(np.float32).reshape(B, DIM, H, W)


if __name__ == "__main__":
    rng = np.random.default_rng(0)
    inputs = {
        "question_rep": rng.standard_normal((B, WDIM + DIM), dtype=np.float32) * 0.05,
        "lhs_rep": rng.standard_normal((B, DIM, H, W), dtype=np.float32),
        "rhs_rep": rng.standard_normal((B, DIM, H, W), dtype=np.float32),
        "proj_w": rng.standard_normal((DIM, 2 * DIM), dtype=np.float32),
        "proj_b": rng.standard_normal((DIM,), dtype=np.float32) * 0.01,
    }
    out = kernel(**inputs)
    print("ran, out shape:", out.shape)


# revision 7
# speedup vs baseline: 1.1409x; 1.0115x over previous
"""Dynamic per-sample CNN (nn_ConvFunc) Trainium2 Bass kernel — bf16 version.

Reference computation (per sample b):
  cnn_inp = proj_w @ cat(lhs, rhs) + proj_b          # 1x1 conv, [128, 32, 32]
  out     = conv3x3(cnn_inp, W_b) + bias_b           # W_b, bias_b unpacked from question_rep[b]

Sharding: pure data parallel, 8 samples per NeuronCore (batch 64 / 8 cores).

v2 changes vs the fp32r baseline (67.9us):
  - all matmul operands bf16 (tolerance 2e-2 >> bf16's ~2e-3): halves HBM
    traffic (17.3MB -> 8.6MB per core), moving the kernel from DMA-bound
    (48us floor) to PE-bound (37.5us floor = 90112 PE cycles @ 2.4GHz).
  - per-sample inputs packed host-side into one [128, 1152+2048] tile
    ([qw | xl | xr]) so loads are few large DMAs; all loads issued up-front
    (SBUF holds all 8 samples), ordered x0,x1,qw0,x2,qw1,... so the PE
    pipeline never starves during ramp-up.
  - dummy warmup matmuls during the initial DMA ramp lift the PE HAM clock
    gate (4/8 -> 8/8 after ~3.4us of activity).
  - conv PSUM eviction split across DVE (half 0) and ACT (half 1); output
    stored as bf16 (host upcasts) halving store traffic and eviction work.
"""

import numpy as np
import ml_dtypes

import concourse.bass as bass
import concourse.mybir as mybir
from concourse import bacc
from concourse.tile import TileContext
from concourse.bass_utils import run_bass_kernel_spmd

# Problem shapes (hardcoded per contract)
B = 64
DIM = 128
H = W = 32
K = 3
KK = K * K
HW = H * W             # 1024
WDIM = DIM * DIM * KK  # 147456
NCORES = 8
SPC = B // NCORES      # samples per core
HP, WP = H + 2, W + 2  # padded 34x34
HALF = HW // 2         # 512 columns per PSUM bank
HROWS = H // 2         # 16 output rows per half
QWC = KK * DIM         # 1152 conv-weight cols per sample
PXC = QWC + 2 * HW     # 3200 packed cols per sample: [qw | xl | xr]

FP = mybir.dt.float32
BF = mybir.dt.bfloat16
BF_NP = ml_dtypes.bfloat16

_BUILT = {}


def build_nc():
    nc = bacc.Bacc("TRN2", target_bir_lowering=False, debug=False,
                   num_devices=NCORES)

    px = nc.declare_dram_parameter("px", [SPC, DIM, PXC], BF, isOutput=False)
    pw = nc.declare_dram_parameter("pw", [DIM, 2 * DIM], BF, isOutput=False)
    bias = nc.declare_dram_parameter("bias", [DIM, SPC + 1], FP, isOutput=False)
    out = nc.declare_dram_parameter("out", [SPC, DIM, HW], BF, isOutput=True)

    with TileContext(nc) as tc:
        with (
            tc.tile_pool(name="const", bufs=1) as cpool,
            tc.tile_pool(name="pxpool", bufs=SPC) as pxpool,
            tc.tile_pool(name="xppool", bufs=4) as xppool,
            tc.tile_pool(name="opool", bufs=4) as opool,
            tc.tile_pool(name="pp_pool", bufs=2, space="PSUM") as pp_pool,
            tc.tile_pool(name="pc_pool", bufs=4, space="PSUM") as pc_pool,
            tc.tile_pool(name="wm_pool", bufs=1, space="PSUM") as wm_pool,
        ):
            pw_sb = cpool.tile([DIM, 2 * DIM], BF)
            bias_sb = cpool.tile([DIM, SPC + 1], FP)
            warm = cpool.tile([DIM, HALF], BF)
            # first DVE instr: gates the PE warmup matmuls, so issue first
            nc.vector.memset(warm[:], 0.0)

            px_sb = [pxpool.tile([DIM, PXC], BF, tag="px", name=f"px{s}")
                     for s in range(SPC)]

            # ---- all loads up-front, split across the two HWDGE rings -----
            # sync (SP) ring: the latency-critical x stream, few big DMAs
            nc.sync.dma_start(out=px_sb[0][:, QWC:PXC], in_=px[0, :, QWC:PXC])
            for s in range(1, SPC):
                nc.sync.dma_start(out=px_sb[s][:], in_=px[s])
            # scalar (ACT) ring: consts + sample-0 conv weights (idle early)
            nc.scalar.dma_start(out=pw_sb[:], in_=pw[:])
            nc.scalar.dma_start(out=bias_sb[:], in_=bias[:])
            nc.scalar.dma_start(out=px_sb[0][:, 0:QWC], in_=px[0, :, 0:QWC])

            # ---- PE warmup: lift the HAM clock gate during the DMA ramp ---
            wmb = wm_pool.tile([DIM, HALF], FP)

            def warmup(n):
                for _ in range(n):
                    nc.tensor.matmul(wmb[:], lhsT=warm[:, 0:DIM], rhs=warm[:],
                                     start=True, stop=True)

            warmup(6)

            pb_ap = bias_sb[:, SPC:SPC + 1]

            def proj(s):
                xp = xppool.tile([DIM, HP, WP], BF, tag="xp")
                if s < 4:
                    # borders only need zeroing once per pool buffer; the
                    # interior is fully rewritten every rotation
                    nc.vector.memset(xp[:, 0:1, :], 0.0)
                    nc.vector.memset(xp[:, HP - 1:HP, :], 0.0)
                    nc.vector.memset(xp[:, 1:HP - 1, 0:1], 0.0)
                    nc.vector.memset(xp[:, 1:HP - 1, WP - 1:WP], 0.0)
                for h in range(2):
                    ppt = pp_pool.tile([DIM, HALF], FP, tag="pp")
                    nc.tensor.matmul(ppt[:], lhsT=pw_sb[:, 0:DIM],
                                     rhs=px_sb[s][:, QWC + h * HALF:
                                                  QWC + (h + 1) * HALF],
                                     start=True, stop=False)
                    nc.tensor.matmul(ppt[:], lhsT=pw_sb[:, DIM:2 * DIM],
                                     rhs=px_sb[s][:, QWC + HW + h * HALF:
                                                  QWC + HW + (h + 1) * HALF],
                                     start=False, stop=True)
                    nc.scalar.activation(
                        xp[:, 1 + HROWS * h:1 + HROWS * (h + 1), 1:1 + W],
                        ppt[:].rearrange("p (a b) -> p a b", b=W),
                        mybir.ActivationFunctionType.Identity,
                        bias=pb_ap,
                    )
                return xp

            def conv(s, xp):
                o_sb = opool.tile([DIM, HW], BF, tag="o")
                pct0 = pc_pool.tile([DIM, HALF], FP, tag="pc")
                pct1 = pc_pool.tile([DIM, HALF], FP, tag="pc")
                pcts = [pct0, pct1]
                qb = bias_sb[:, s:s + 1]
                if s == SPC - 1:
                    # h-outer for the last sample: half 0 evicts + stores
                    # while half 1's taps still run, shortening the tail
                    for h in range(2):
                        for t in range(KK):
                            kh, kw = divmod(t, K)
                            nc.tensor.matmul(
                                pcts[h][:],
                                lhsT=px_sb[s][:, t * DIM:(t + 1) * DIM],
                                rhs=xp[:, HROWS * h + kh:HROWS * (h + 1) + kh,
                                       kw:kw + W],
                                start=(t == 0), stop=(t == KK - 1))
                        nc.vector.tensor_scalar_add(
                            o_sb[:, h * HALF:(h + 1) * HALF], pcts[h][:], qb)
                        nc.scalar.dma_start(
                            out=out[s, :, h * HALF:(h + 1) * HALF],
                            in_=o_sb[:, h * HALF:(h + 1) * HALF])
                    return
                # tap-outer: consecutive matmuls share the stationary weights
                for t in range(KK):
                    kh, kw = divmod(t, K)
                    for h in range(2):
                        nc.tensor.matmul(
                            pcts[h][:],
                            lhsT=px_sb[s][:, t * DIM:(t + 1) * DIM],
                            rhs=xp[:, HROWS * h + kh:HROWS * (h + 1) + kh,
                                   kw:kw + W],
                            start=(t == 0), stop=(t == KK - 1))
                # evict half 0 on DVE, half 1 on ACT; bf16 out halves traffic
                nc.vector.tensor_scalar_add(o_sb[:, 0:HALF], pct0[:], qb)
                nc.scalar.activation(
                    o_sb[:, HALF:HW], pct1[:],
                    mybir.ActivationFunctionType.Identity, bias=qb)
                nc.scalar.dma_start(out=out[s], in_=o_sb[:])

            # software pipeline: proj(s) ahead of conv(s-1) keeps PE dense;
            # early warmup matmuls fill DMA-ramp stalls and keep the HAM
            # activity window busy so the clock un-gates at ~10us not ~21us
            prev = None
            for s in range(SPC):
                if s == 1:
                    warmup(2)
                xp = proj(s)
                if prev is not None:
                    conv(*prev)
                if s == 1:
                    warmup(1)
                prev = (s, xp)
            conv(*prev)

    nc.compile()
    return nc


def _prep(question_rep, lhs_rep, rhs_rep, proj_w, proj_b):
    """Host-side shard + layout prep (reshape/transpose + bf16 cast)."""
    qr = np.ascontiguousarray(question_rep, dtype=np.float32)
    # conv weights: [B, o, i, kh, kw] -> [B, i, (kh kw), o] so each tap is a
    # ready lhsT [i, o] block
    qw = qr[:, :WDIM].reshape(B, DIM, DIM, K, K).transpose(0, 2, 3, 4, 1)
    qw = np.ascontiguousarray(qw).reshape(B, DIM, QWC)
    qb = np.ascontiguousarray(qr[:, WDIM:])             # [B, 128]
    xl = np.asarray(lhs_rep, dtype=np.float32).reshape(B, DIM, HW)
    xr = np.asarray(rhs_rep, dtype=np.float32).reshape(B, DIM, HW)
    pxf = np.concatenate([qw, xl, xr], axis=2).astype(BF_NP)  # [B, 128, 3200]
    pwt = np.asarray(proj_w, dtype=np.float32).T        # [256, 128]
    pw_h = np.concatenate([pwt[:DIM], pwt[DIM:]], axis=1).astype(BF_NP)
    pb = np.asarray(proj_b, dtype=np.float32).reshape(DIM, 1)

    in_maps = []
    for c in range(NCORES):
        sl = slice(c * SPC, (c + 1) * SPC)
        biasm = np.concatenate([qb[sl].T, pb], axis=1).astype(np.float32)
        in_maps.append({
            "px": np.ascontiguousarray(pxf[sl]),
            "pw": pw_h,
            "bias": np.ascontiguousarray(biasm),
        })
    return in_maps


def kernel(question_rep, lhs_rep, rhs_rep, proj_w, proj_b, _run_kwargs=None):
    if "nc" not in _BUILT:
        _BUILT["nc"] = build_nc()
    nc = _BUILT["nc"]
    in_maps = _prep(question_rep, lhs_rep, rhs_rep, proj_w, proj_b)
    res = run_bass_kernel_spmd(nc, in_maps, core_ids=list(range(NCORES)),
                               **(_run_kwargs or {}))
    out = np.concatenate([np.asarray(res.results[c]["out"])
                          for c in range(NCORES)], axis=0)
    if _run_kwargs is not None:
        _BUILT["last_result"] = res
    return out.astype(np.float32).reshape(B, DIM, H, W)


if __name__ == "__main__":
    rng = np.random.default_rng(0)
    inputs = {
        "question_rep": rng.standard_normal((B, WDIM + DIM), dtype=np.float32) * 0.05,
        "lhs_rep": rng.standard_normal((B, DIM, H, W), dtype=np.float32),
        "rhs_rep": rng.standard_normal((B, DIM, H, W), dtype=np.float32),
        "proj_w": rng.standard_normal((DIM, 2 * DIM), dtype=np.float32),
        "proj_b": rng.standard_normal((DIM,), dtype=np.float32) * 0.01,
    }
    out = kernel(**inputs)
    print("ran, out shape:", out.shape)


# revision 8
# speedup vs baseline: 1.2281x; 1.0764x over previous
"""Dynamic per-sample CNN (nn_ConvFunc) Trainium2 Bass kernel — bf16 version.

Reference computation (per sample b):
  cnn_inp = proj_w @ cat(lhs, rhs) + proj_b          # 1x1 conv, [128, 32, 32]
  out     = conv3x3(cnn_inp, W_b) + bias_b           # W_b, bias_b unpacked from question_rep[b]

Sharding: pure data parallel, 8 samples per NeuronCore (batch 64 / 8 cores).

Design (v4):
  - all matmul operands bf16 (tolerance 2e-2 >> bf16's ~3e-3): fp32 matmul
    streams at half rate on the trn2 PE, so bf16 halves PE time (90112
    columns -> 37.5us @ 2.4GHz) and halves HBM traffic.
  - per-sample inputs packed host-side into [128, 1152+2048] ([qw | xl | xr]);
    loads split x-part/w-part and issued up-front on the sync HWDGE ring in
    exact consumption order (pwb, x0, x1, w0, x2, w1, ...) — each DMA_DIRECT2D
    issue costs ~650ns of sequencer time, so few big DMAs win.
  - proj/conv biases ride inside the pwb param as fp32 bit-pattern pairs of
    bf16 columns (bitcast on device): no tiny-descriptor bias DMA (a 36B/
    partition DMA starved behind the px stream cost 7us in v3).
  - dummy warmup matmuls fill the DMA ramp and known early stalls so the PE
    HAM clock gate lifts at ~11us and never re-throttles (idle >3.4us would
    drop the PE clock 2.4 -> 1.2 GHz).
  - PSUM eviction split across DVE (conv h0) and ACT (proj, conv h1); output
    stored bf16 (host upcasts). Last sample runs h-outer with its second half
    column-chunked so the final evict+store tail is ~256 cols, not 1024.
"""

import numpy as np
import ml_dtypes

import concourse.bass as bass
import concourse.mybir as mybir
from concourse import bacc
from concourse.tile import TileContext
from concourse.bass_utils import run_bass_kernel_spmd

# Problem shapes (hardcoded per contract)
B = 64
DIM = 128
H = W = 32
K = 3
KK = K * K
HW = H * W             # 1024
WDIM = DIM * DIM * KK  # 147456
NCORES = 8
SPC = B // NCORES      # samples per core
HP, WP = H + 2, W + 2  # padded 34x34
HALF = HW // 2         # 512 columns per PSUM bank
QUART = HALF // 2      # 256-col tail chunks
HROWS = H // 2         # 16 output rows per half
QROWS = HROWS // 2     # 8 rows per tail chunk
QWC = KK * DIM         # 1152 conv-weight cols per sample
PXC = QWC + 2 * HW     # 3200 packed cols per sample: [qw | xl | xr]
PWC = 2 * DIM + 2 * (SPC + 1)  # pw cols + bitcast fp32 bias cols (18)

FP = mybir.dt.float32
BF = mybir.dt.bfloat16
BF_NP = ml_dtypes.bfloat16

_BUILT = {}


def build_nc():
    nc = bacc.Bacc("TRN2", target_bir_lowering=False, debug=False,
                   num_devices=NCORES)

    px = nc.declare_dram_parameter("px", [SPC, DIM, PXC], BF, isOutput=False)
    pwb = nc.declare_dram_parameter("pwb", [DIM, PWC], BF, isOutput=False)
    out = nc.declare_dram_parameter("out", [SPC, DIM, HW], BF, isOutput=True)

    with TileContext(nc) as tc:
        with (
            tc.tile_pool(name="const", bufs=1) as cpool,
            tc.tile_pool(name="pxpool", bufs=SPC) as pxpool,
            tc.tile_pool(name="xppool", bufs=4) as xppool,
            tc.tile_pool(name="opool", bufs=4) as opool,
            tc.tile_pool(name="pp_pool", bufs=2, space="PSUM") as pp_pool,
            tc.tile_pool(name="pc_pool", bufs=4, space="PSUM") as pc_pool,
            tc.tile_pool(name="wm_pool", bufs=1, space="PSUM") as wm_pool,
        ):
            pwb_sb = cpool.tile([DIM, PWC], BF)
            warm = cpool.tile([DIM, HALF], BF)
            # first DVE instr: gates the PE warmup matmuls, so issue first
            nc.vector.memset(warm[:], 0.0)

            px_sb = [pxpool.tile([DIM, PXC], BF, tag="px", name=f"px{s}")
                     for s in range(SPC)]

            # ---- all loads up-front on the sync ring, consumption order ---
            def load_x(s):
                nc.sync.dma_start(out=px_sb[s][:, QWC:PXC],
                                  in_=px[s, :, QWC:PXC])

            def load_w(s):
                nc.sync.dma_start(out=px_sb[s][:, 0:QWC], in_=px[s, :, 0:QWC])

            nc.sync.dma_start(out=pwb_sb[:], in_=pwb[:])
            load_x(0)
            load_x(1)
            load_w(0)
            for s in range(2, SPC):
                load_x(s)
                load_w(s - 1)
            load_w(SPC - 1)

            # ---- PE warmup: lift the HAM clock gate during the DMA ramp ---
            wmb = wm_pool.tile([DIM, HALF], FP)

            def warmup(n):
                for _ in range(n):
                    nc.tensor.matmul(wmb[:], lhsT=warm[:, 0:DIM], rhs=warm[:],
                                     start=True, stop=True)

            warmup(7)

            pw0 = pwb_sb[:, 0:DIM]
            pw1 = pwb_sb[:, DIM:2 * DIM]

            def qb_ap(s):
                o = 2 * DIM + 2 * s
                return pwb_sb[:, o:o + 2].bitcast(FP)

            pb_ap = pwb_sb[:, 2 * DIM + 2 * SPC:PWC].bitcast(FP)

            def proj(s):
                xp = xppool.tile([DIM, HP, WP], BF, tag="xp")
                if s < 4:
                    # borders only need zeroing once per pool buffer; the
                    # interior is fully rewritten every rotation
                    nc.vector.memset(xp[:, 0:1, :], 0.0)
                    nc.vector.memset(xp[:, HP - 1:HP, :], 0.0)
                    nc.vector.memset(xp[:, 1:HP - 1, 0:1], 0.0)
                    nc.vector.memset(xp[:, 1:HP - 1, WP - 1:WP], 0.0)
                for h in range(2):
                    ppt = pp_pool.tile([DIM, HALF], FP, tag="pp")
                    nc.tensor.matmul(ppt[:], lhsT=pw0,
                                     rhs=px_sb[s][:, QWC + h * HALF:
                                                  QWC + (h + 1) * HALF],
                                     start=True, stop=False)
                    nc.tensor.matmul(ppt[:], lhsT=pw1,
                                     rhs=px_sb[s][:, QWC + HW + h * HALF:
                                                  QWC + HW + (h + 1) * HALF],
                                     start=False, stop=True)
                    nc.scalar.activation(
                        xp[:, 1 + HROWS * h:1 + HROWS * (h + 1), 1:1 + W],
                        ppt[:].rearrange("p (a b) -> p a b", b=W),
                        mybir.ActivationFunctionType.Identity,
                        bias=pb_ap,
                    )
                return xp

            def wtap(s, t):
                return px_sb[s][:, t * DIM:(t + 1) * DIM]

            def conv(s, xp):
                o_sb = opool.tile([DIM, HW], BF, tag="o")
                pct0 = pc_pool.tile([DIM, HALF], FP, tag="pc")
                pct1 = pc_pool.tile([DIM, HALF], FP, tag="pc")
                pcts = [pct0, pct1]
                qb = qb_ap(s)
                # tap-outer: consecutive matmuls share the stationary weights
                for t in range(KK):
                    kh, kw = divmod(t, K)
                    for h in range(2):
                        nc.tensor.matmul(
                            pcts[h][:],
                            lhsT=wtap(s, t),
                            rhs=xp[:, HROWS * h + kh:HROWS * (h + 1) + kh,
                                   kw:kw + W],
                            start=(t == 0), stop=(t == KK - 1))
                # evict half 0 on DVE, half 1 on ACT; bf16 out halves traffic
                nc.vector.tensor_scalar_add(o_sb[:, 0:HALF], pct0[:], qb)
                nc.scalar.activation(
                    o_sb[:, HALF:HW], pct1[:],
                    mybir.ActivationFunctionType.Identity, bias=qb)
                nc.scalar.dma_start(out=out[s], in_=o_sb[:])

            def conv_last(s, xp):
                # h-outer + column-chunked second half: each chunk evicts and
                # stores while later chunks' taps still run -> short tail
                o_sb = opool.tile([DIM, HW], BF, tag="o")
                qb = qb_ap(s)
                pct0 = pc_pool.tile([DIM, HALF], FP, tag="pc")
                for t in range(KK):
                    kh, kw = divmod(t, K)
                    nc.tensor.matmul(
                        pct0[:], lhsT=wtap(s, t),
                        rhs=xp[:, kh:HROWS + kh, kw:kw + W],
                        start=(t == 0), stop=(t == KK - 1))
                nc.vector.tensor_scalar_add(o_sb[:, 0:HALF], pct0[:], qb)
                nc.scalar.dma_start(out=out[s, :, 0:HALF],
                                    in_=o_sb[:, 0:HALF])
                for c in range(2):
                    pcq = pc_pool.tile([DIM, HALF], FP, tag="pc")
                    r0 = HROWS + QROWS * c
                    c0 = HALF + QUART * c
                    for t in range(KK):
                        kh, kw = divmod(t, K)
                        nc.tensor.matmul(
                            pcq[:, 0:QUART], lhsT=wtap(s, t),
                            rhs=xp[:, r0 + kh:r0 + QROWS + kh, kw:kw + W],
                            start=(t == 0), stop=(t == KK - 1))
                    nc.vector.tensor_scalar_add(
                        o_sb[:, c0:c0 + QUART], pcq[:, 0:QUART], qb)
                    nc.scalar.dma_start(out=out[s, :, c0:c0 + QUART],
                                        in_=o_sb[:, c0:c0 + QUART])

            # software pipeline: proj(s) ahead of conv(s-1) keeps PE dense;
            # warmup matmuls fill the known early DMA-ramp stalls
            prev = None
            for s in range(SPC):
                if s == 1:
                    warmup(1)
                xp = proj(s)
                if s == 1:
                    warmup(2)
                if prev is not None:
                    if prev[0] == SPC - 1:
                        conv_last(*prev)
                    else:
                        conv(*prev)
                if s == 1:
                    warmup(1)
                prev = (s, xp)
            conv_last(*prev)

    nc.compile()
    return nc


def _prep(question_rep, lhs_rep, rhs_rep, proj_w, proj_b):
    """Host-side shard + layout prep (reshape/transpose + bf16 cast)."""
    qr = np.ascontiguousarray(question_rep, dtype=np.float32)
    # conv weights: [B, o, i, kh, kw] -> [B, i, (kh kw), o] so each tap is a
    # ready lhsT [i, o] block
    qw = qr[:, :WDIM].reshape(B, DIM, DIM, K, K).transpose(0, 2, 3, 4, 1)
    qw = np.ascontiguousarray(qw).reshape(B, DIM, QWC)
    qb = np.ascontiguousarray(qr[:, WDIM:])             # [B, 128]
    xl = np.asarray(lhs_rep, dtype=np.float32).reshape(B, DIM, HW)
    xr = np.asarray(rhs_rep, dtype=np.float32).reshape(B, DIM, HW)
    pxf = np.concatenate([qw, xl, xr], axis=2).astype(BF_NP)  # [B, 128, 3200]
    pwt = np.asarray(proj_w, dtype=np.float32).T        # [256, 128]
    pw_h = np.concatenate([pwt[:DIM], pwt[DIM:]], axis=1).astype(BF_NP)
    pb = np.asarray(proj_b, dtype=np.float32).reshape(DIM, 1)

    in_maps = []
    for c in range(NCORES):
        sl = slice(c * SPC, (c + 1) * SPC)
        # biases ride as fp32 bit-patterns in bf16 columns (device bitcasts)
        biasm = np.concatenate([qb[sl].T, pb], axis=1).astype(np.float32)
        bias_bf = np.ascontiguousarray(biasm).view(np.uint16).view(BF_NP)
        pwbm = np.concatenate([pw_h, bias_bf], axis=1)  # [128, 274] bf16
        in_maps.append({
            "px": np.ascontiguousarray(pxf[sl]),
            "pwb": np.ascontiguousarray(pwbm),
        })
    return in_maps


def kernel(question_rep, lhs_rep, rhs_rep, proj_w, proj_b, _run_kwargs=None):
    if "nc" not in _BUILT:
        _BUILT["nc"] = build_nc()
    nc = _BUILT["nc"]
    in_maps = _prep(question_rep, lhs_rep, rhs_rep, proj_w, proj_b)
    res = run_bass_kernel_spmd(nc, in_maps, core_ids=list(range(NCORES)),
                               **(_run_kwargs or {}))
    out = np.concatenate([np.asarray(res.results[c]["out"])
                          for c in range(NCORES)], axis=0)
    if _run_kwargs is not None:
        _BUILT["last_result"] = res
    return out.astype(np.float32).reshape(B, DIM, H, W)


if __name__ == "__main__":
    rng = np.random.default_rng(0)
    inputs = {
        "question_rep": rng.standard_normal((B, WDIM + DIM), dtype=np.float32) * 0.05,
        "lhs_rep": rng.standard_normal((B, DIM, H, W), dtype=np.float32),
        "rhs_rep": rng.standard_normal((B, DIM, H, W), dtype=np.float32),
        "proj_w": rng.standard_normal((DIM, 2 * DIM), dtype=np.float32),
        "proj_b": rng.standard_normal((DIM,), dtype=np.float32) * 0.01,
    }
    out = kernel(**inputs)
    print("ran, out shape:", out.shape)


# revision 12
# speedup vs baseline: 1.2474x; 1.0157x over previous
"""Dynamic per-sample CNN (nn_ConvFunc) Trainium2 Bass kernel — bf16 version.

Reference computation (per sample b):
  cnn_inp = proj_w @ cat(lhs, rhs) + proj_b          # 1x1 conv, [128, 32, 32]
  out     = conv3x3(cnn_inp, W_b) + bias_b           # W_b, bias_b unpacked from question_rep[b]

Sharding: pure data parallel, 8 samples per NeuronCore (batch 64 / 8 cores).

Design (v4):
  - all matmul operands bf16 (tolerance 2e-2 >> bf16's ~3e-3): fp32 matmul
    streams at half rate on the trn2 PE, so bf16 halves PE time (90112
    columns -> 37.5us @ 2.4GHz) and halves HBM traffic.
  - per-sample inputs packed host-side into [128, 1152+2048] ([qw | xl | xr]);
    loads split x-part/w-part and issued up-front on the sync HWDGE ring in
    exact consumption order (pwb, x0, x1, w0, x2, w1, ...) — each DMA_DIRECT2D
    issue costs ~650ns of sequencer time, so few big DMAs win.
  - proj/conv biases ride inside the pwb param as fp32 bit-pattern pairs of
    bf16 columns (bitcast on device): no tiny-descriptor bias DMA (a 36B/
    partition DMA starved behind the px stream cost 7us in v3).
  - dummy warmup matmuls fill the DMA ramp and known early stalls so the PE
    HAM clock gate lifts at ~11us and never re-throttles (idle >3.4us would
    drop the PE clock 2.4 -> 1.2 GHz).
  - PSUM eviction split across DVE (conv h0) and ACT (proj, conv h1); output
    stored bf16 (host upcasts). Last sample runs h-outer with its second half
    column-chunked so the final evict+store tail is ~256 cols, not 1024.
"""

import numpy as np
import ml_dtypes

import concourse.bass as bass
import concourse.mybir as mybir
from concourse import bacc
from concourse.tile import TileContext
from concourse.bass_utils import run_bass_kernel_spmd

# Problem shapes (hardcoded per contract)
B = 64
DIM = 128
H = W = 32
K = 3
KK = K * K
HW = H * W             # 1024
WDIM = DIM * DIM * KK  # 147456
NCORES = 8
SPC = B // NCORES      # samples per core
HP, WP = H + 2, W + 2  # padded 34x34
HALF = HW // 2         # 512 columns per PSUM bank
QUART = HALF // 2      # 256-col tail chunks
HROWS = H // 2         # 16 output rows per half
QROWS = HROWS // 2     # 8 rows per tail chunk
QWC = KK * DIM         # 1152 conv-weight cols per sample
PXC = QWC + 2 * HW     # 3200 packed cols per sample: [qw | xl | xr]
PWC = 2 * DIM + 2 * (SPC + 1)  # pw cols + bitcast fp32 bias cols (18)

FP = mybir.dt.float32
BF = mybir.dt.bfloat16
BF_NP = ml_dtypes.bfloat16

_BUILT = {}


def build_nc():
    nc = bacc.Bacc("TRN2", target_bir_lowering=False, debug=False,
                   num_devices=NCORES)

    px = nc.declare_dram_parameter("px", [SPC, DIM, PXC], BF, isOutput=False)
    pwb = nc.declare_dram_parameter("pwb", [DIM, PWC], BF, isOutput=False)
    out = nc.declare_dram_parameter("out", [SPC, DIM, HW], BF, isOutput=True)

    with TileContext(nc) as tc:
        with (
            tc.tile_pool(name="const", bufs=1) as cpool,
            tc.tile_pool(name="pxpool", bufs=SPC) as pxpool,
            tc.tile_pool(name="xppool", bufs=4) as xppool,
            tc.tile_pool(name="opool", bufs=4) as opool,
            tc.tile_pool(name="pp_pool", bufs=2, space="PSUM") as pp_pool,
            tc.tile_pool(name="pc_pool", bufs=4, space="PSUM") as pc_pool,
            tc.tile_pool(name="wm_pool", bufs=1, space="PSUM") as wm_pool,
        ):
            pwb_sb = cpool.tile([DIM, PWC], BF)
            warm = cpool.tile([DIM, HALF], BF)
            # gates the PE warmup matmuls; gpsimd reaches its first user
            # instruction earliest and is otherwise idle
            nc.gpsimd.memset(warm[:], 0.0)

            px_sb = [pxpool.tile([DIM, PXC], BF, tag="px", name=f"px{s}")
                     for s in range(SPC)]

            # ---- all loads up-front on the sync ring, consumption order ---
            def load_x(s):
                nc.sync.dma_start(out=px_sb[s][:, QWC:PXC],
                                  in_=px[s, :, QWC:PXC])

            def load_w(s):
                nc.sync.dma_start(out=px_sb[s][:, 0:QWC], in_=px[s, :, 0:QWC])

            # consumption order: conv(s) needs w(s) right after proj(s+1)
            # needs x(s+1), so pair them w-then-x
            nc.sync.dma_start(out=pwb_sb[:], in_=pwb[:])
            load_x(0)
            load_x(1)
            for s in range(SPC - 2):
                load_w(s)
                load_x(s + 2)
            load_w(SPC - 2)
            load_w(SPC - 1)

            # ---- PE warmup: lift the HAM clock gate during the DMA ramp ---
            wmb = wm_pool.tile([DIM, HALF], FP)

            def warmup(n):
                for _ in range(n):
                    nc.tensor.matmul(wmb[:], lhsT=warm[:, 0:DIM], rhs=warm[:],
                                     start=True, stop=True)

            warmup(9)

            pw0 = pwb_sb[:, 0:DIM]
            pw1 = pwb_sb[:, DIM:2 * DIM]

            def qb_ap(s):
                o = 2 * DIM + 2 * s
                return pwb_sb[:, o:o + 2].bitcast(FP)

            pb_ap = pwb_sb[:, 2 * DIM + 2 * SPC:PWC].bitcast(FP)

            def proj(s):
                xp = xppool.tile([DIM, HP, WP], BF, tag="xp")
                if s < 4:
                    # borders only need zeroing once per pool buffer; the
                    # interior is fully rewritten every rotation
                    nc.vector.memset(xp[:, 0:1, :], 0.0)
                    nc.vector.memset(xp[:, HP - 1:HP, :], 0.0)
                    nc.vector.memset(xp[:, 1:HP - 1, 0:1], 0.0)
                    nc.vector.memset(xp[:, 1:HP - 1, WP - 1:WP], 0.0)
                for h in range(2):
                    ppt = pp_pool.tile([DIM, HALF], FP, tag="pp")
                    nc.tensor.matmul(ppt[:], lhsT=pw0,
                                     rhs=px_sb[s][:, QWC + h * HALF:
                                                  QWC + (h + 1) * HALF],
                                     start=True, stop=False)
                    nc.tensor.matmul(ppt[:], lhsT=pw1,
                                     rhs=px_sb[s][:, QWC + HW + h * HALF:
                                                  QWC + HW + (h + 1) * HALF],
                                     start=False, stop=True)
                    dst = xp[:, 1 + HROWS * h:1 + HROWS * (h + 1), 1:1 + W]
                    src = ppt[:].rearrange("p (a b) -> p a b", b=W)
                    if h == 0:
                        # h0 on DVE, h1 on ACT: parallel evictions unblock
                        # the dependent conv two engine-ops sooner
                        nc.vector.tensor_scalar_add(dst, src, pb_ap)
                    else:
                        nc.scalar.activation(
                            dst, src,
                            mybir.ActivationFunctionType.Identity,
                            bias=pb_ap,
                        )
                return xp

            def wtap(s, t):
                return px_sb[s][:, t * DIM:(t + 1) * DIM]

            def conv(s, xp):
                o_sb = opool.tile([DIM, HW], BF, tag="o")
                pct0 = pc_pool.tile([DIM, HALF], FP, tag="pc")
                pct1 = pc_pool.tile([DIM, HALF], FP, tag="pc")
                pcts = [pct0, pct1]
                qb = qb_ap(s)
                # tap-outer: consecutive matmuls share the stationary weights
                for t in range(KK):
                    kh, kw = divmod(t, K)
                    for h in range(2):
                        nc.tensor.matmul(
                            pcts[h][:],
                            lhsT=wtap(s, t),
                            rhs=xp[:, HROWS * h + kh:HROWS * (h + 1) + kh,
                                   kw:kw + W],
                            start=(t == 0), stop=(t == KK - 1))
                # evict half 0 on DVE, half 1 on ACT; bf16 out halves traffic
                nc.vector.tensor_scalar_add(o_sb[:, 0:HALF], pct0[:], qb)
                nc.scalar.activation(
                    o_sb[:, HALF:HW], pct1[:],
                    mybir.ActivationFunctionType.Identity, bias=qb)
                nc.scalar.dma_start(out=out[s], in_=o_sb[:])

            def conv_last(s, xp):
                # h-outer + column-chunked second half: each chunk evicts and
                # stores while later chunks' taps still run -> short tail
                o_sb = opool.tile([DIM, HW], BF, tag="o")
                qb = qb_ap(s)
                pct0 = pc_pool.tile([DIM, HALF], FP, tag="pc")
                for t in range(KK):
                    kh, kw = divmod(t, K)
                    nc.tensor.matmul(
                        pct0[:], lhsT=wtap(s, t),
                        rhs=xp[:, kh:HROWS + kh, kw:kw + W],
                        start=(t == 0), stop=(t == KK - 1))
                nc.vector.tensor_scalar_add(o_sb[:, 0:HALF], pct0[:], qb)
                nc.scalar.dma_start(out=out[s, :, 0:HALF],
                                    in_=o_sb[:, 0:HALF])
                for c in range(2):
                    pcq = pc_pool.tile([DIM, HALF], FP, tag="pc")
                    r0 = HROWS + QROWS * c
                    c0 = HALF + QUART * c
                    for t in range(KK):
                        kh, kw = divmod(t, K)
                        nc.tensor.matmul(
                            pcq[:, 0:QUART], lhsT=wtap(s, t),
                            rhs=xp[:, r0 + kh:r0 + QROWS + kh, kw:kw + W],
                            start=(t == 0), stop=(t == KK - 1))
                    nc.vector.tensor_scalar_add(
                        o_sb[:, c0:c0 + QUART], pcq[:, 0:QUART], qb)
                    nc.scalar.dma_start(out=out[s, :, c0:c0 + QUART],
                                        in_=o_sb[:, c0:c0 + QUART])

            # software pipeline: proj(s) ahead of conv(s-1) keeps PE dense;
            # warmup matmuls fill the known early DMA-ramp stalls
            prev = None
            for s in range(SPC):
                if s == 1:
                    warmup(1)
                xp = proj(s)
                if s == 1:
                    warmup(2)
                if prev is not None:
                    if prev[0] == SPC - 1:
                        conv_last(*prev)
                    else:
                        conv(*prev)
                if s == 1:
                    warmup(1)
                prev = (s, xp)
            conv_last(*prev)

    nc.compile()
    return nc


def _prep(question_rep, lhs_rep, rhs_rep, proj_w, proj_b):
    """Host-side shard + layout prep (reshape/transpose + bf16 cast)."""
    qr = np.ascontiguousarray(question_rep, dtype=np.float32)
    # conv weights: [B, o, i, kh, kw] -> [B, i, (kh kw), o] so each tap is a
    # ready lhsT [i, o] block
    qw = qr[:, :WDIM].reshape(B, DIM, DIM, K, K).transpose(0, 2, 3, 4, 1)
    qw = np.ascontiguousarray(qw).reshape(B, DIM, QWC)
    qb = np.ascontiguousarray(qr[:, WDIM:])             # [B, 128]
    xl = np.asarray(lhs_rep, dtype=np.float32).reshape(B, DIM, HW)
    xr = np.asarray(rhs_rep, dtype=np.float32).reshape(B, DIM, HW)
    pxf = np.concatenate([qw, xl, xr], axis=2).astype(BF_NP)  # [B, 128, 3200]
    pwt = np.asarray(proj_w, dtype=np.float32).T        # [256, 128]
    pw_h = np.concatenate([pwt[:DIM], pwt[DIM:]], axis=1).astype(BF_NP)
    pb = np.asarray(proj_b, dtype=np.float32).reshape(DIM, 1)

    in_maps = []
    for c in range(NCORES):
        sl = slice(c * SPC, (c + 1) * SPC)
        # biases ride as fp32 bit-patterns in bf16 columns (device bitcasts)
        biasm = np.concatenate([qb[sl].T, pb], axis=1).astype(np.float32)
        bias_bf = np.ascontiguousarray(biasm).view(np.uint16).view(BF_NP)
        pwbm = np.concatenate([pw_h, bias_bf], axis=1)  # [128, 274] bf16
        in_maps.append({
            "px": np.ascontiguousarray(pxf[sl]),
            "pwb": np.ascontiguousarray(pwbm),
        })
    return in_maps


def kernel(question_rep, lhs_rep, rhs_rep, proj_w, proj_b, _run_kwargs=None):
    if "nc" not in _BUILT:
        _BUILT["nc"] = build_nc()
    nc = _BUILT["nc"]
    in_maps = _prep(question_rep, lhs_rep, rhs_rep, proj_w, proj_b)
    res = run_bass_kernel_spmd(nc, in_maps, core_ids=list(range(NCORES)),
                               **(_run_kwargs or {}))
    out = np.concatenate([np.asarray(res.results[c]["out"])
                          for c in range(NCORES)], axis=0)
    if _run_kwargs is not None:
        _BUILT["last_result"] = res
    return out.astype(np.float32).reshape(B, DIM, H, W)


if __name__ == "__main__":
    rng = np.random.default_rng(0)
    inputs = {
        "question_rep": rng.standard_normal((B, WDIM + DIM), dtype=np.float32) * 0.05,
        "lhs_rep": rng.standard_normal((B, DIM, H, W), dtype=np.float32),
        "rhs_rep": rng.standard_normal((B, DIM, H, W), dtype=np.float32),
        "proj_w": rng.standard_normal((DIM, 2 * DIM), dtype=np.float32),
        "proj_b": rng.standard_normal((DIM,), dtype=np.float32) * 0.01,
    }
    out = kernel(**inputs)
    print("ran, out shape:", out.shape)
